# revision 22
# baseline (speedup 1.0000x reference)
"""Trainium2 Bass kernel for nn_MultiHeadAttention (B=2, S=2048, H=1024, 16 heads).

Sharding: 8 cores = 2 (batch) x 4 (head-groups of 4 heads). Each core computes
QKV projections for its 256-dim head slice, attention for its 4 heads, and a
partial output projection. Host sums the 4 head-group partials per batch and
adds the output bias.

On-chip layout: activations live transposed as [d, s] with the hidden/head dim
on partitions, so every matmul contraction runs on the PE partition axis with
no activation transposes (inputs are pre-transposed on the host during
sharding). Attention uses unnormalized exp scores with a fused ones-column in
V to produce row sums, normalizing the small [64, S] per-head output instead
of the [S, S] attention matrix.

Schedule: the steady-state attention loop is gated by the scalar engine's exp
throughput, and the PE HAM governor halves the PE clock whenever the PE
micro-idles. So (1) within each group the trailing attn@V and any filler
matmuls are emitted BEFORE the scores matmuls, so the in-order PE queue always
has eligible work while the scores wait on the exp PSUM rotation, and (2) the
V projection, V' transposes and per-q-block Q projections are interleaved into
the attention stream as PE filler instead of running as separate phases (this
also hides the input DMA: scores only need K and the first Q block). Small
copies run on the otherwise-idle Pool engine; softmax normalization uses the
single-pass reciprocal_approx_fast and an fp16 K=1 broadcast matmul.
"""

import sys

if "/opt/trn_rl_repo" not in sys.path:
    sys.path.insert(0, "/opt/trn_rl_repo")

import numpy as np

HIDDEN, HEADS, D_K, B, S = 1024, 16, 64, 2, 2048
G = 4              # head groups (tensor-parallel dim)
HPG = HEADS // G   # heads per group
DSL = HPG * D_K    # 256: d-slice per core
P = 128
QB = 512           # q-block size for attention tiling
N_QB = S // QB     # 4
KC = S // P        # 16 k-chunks
NG = KC // 2       # 8 two-chunk groups
CC = HIDDEN // P   # 8 contraction chunks for projections
SCALE = 1.0 / np.sqrt(np.float32(D_K))


def _build_nc():
    from contextlib import ExitStack

    import concourse.mybir as mybir
    import concourse.tile as tile
    from concourse.bacc import Bacc
    from concourse.masks import make_identity

    dt = mybir.dt
    f32 = dt.float32
    f16 = dt.float16

    nc = Bacc(None)

    qT_d = nc.dram_tensor("qT", [HIDDEN, S], f16, kind="ExternalInput")
    kT_d = nc.dram_tensor("kT", [HIDDEN, S], f16, kind="ExternalInput")
    vT_d = nc.dram_tensor("vT", [HIDDEN, S], f16, kind="ExternalInput")
    wqT_d = nc.dram_tensor("wqT", [HIDDEN, DSL], f16, kind="ExternalInput")
    wkT_d = nc.dram_tensor("wkT", [HIDDEN, DSL], f16, kind="ExternalInput")
    wvT_d = nc.dram_tensor("wvT", [HIDDEN, DSL], f16, kind="ExternalInput")
    woT_d = nc.dram_tensor("woT", [DSL, HIDDEN], f16, kind="ExternalInput")
    bq_d = nc.dram_tensor("bq", [DSL], f32, kind="ExternalInput")
    bk_d = nc.dram_tensor("bk", [DSL], f32, kind="ExternalInput")
    bv_d = nc.dram_tensor("bv", [DSL], f32, kind="ExternalInput")
    y_d = nc.dram_tensor("y", [S, HIDDEN], f16, kind="ExternalOutput")
    y_r = y_d.rearrange("(sc p) e -> p sc e", p=P)

    w_r = {
        "q": wqT_d.rearrange("(c p) d -> p c d", p=P),
        "k": wkT_d.rearrange("(c p) d -> p c d", p=P),
        "v": wvT_d.rearrange("(c p) d -> p c d", p=P),
    }
    b_r = {"q": bq_d, "k": bk_d, "v": bv_d}
    x_r = {
        "q": qT_d.rearrange("(c p) s -> p c s", p=P),
        "k": kT_d.rearrange("(c p) s -> p c s", p=P),
        "v": vT_d.rearrange("(c p) s -> p c s", p=P),
    }

    with tile.TileContext(nc) as tc:
        with (
            tc.tile_pool(name="weights", bufs=1) as wpool,
            tc.tile_pool(name="qkvT", bufs=1) as qkvT_pool,
            tc.tile_pool(name="xT_out", bufs=1) as xT_pool,
            tc.tile_pool(name="small", bufs=1) as small,
        ):
            nc.scalar.add_instruction(
                mybir.InstLoadActFuncSet(
                    name=nc.get_next_instruction_name(),
                    ins=[],
                    outs=[],
                    act_func_set_id=6,  # natural_log_exp_and_others
                )
            )
            ident = small.tile([P, P], f16)
            make_identity(nc, ident)
            ones16 = small.tile([P, D_K], f16, tag="ones")
            nc.vector.memset(ones16[:], 1.0)
            ones32 = small.tile([P, D_K], f32, tag="ones32")
            nc.vector.memset(ones32[:], 1.0)

            proj_out = {}
            for name in ("k", "v", "q"):
                proj_out[name] = qkvT_pool.tile(
                    [P, DSL // P, S], f16, tag=f"{name}T", name=f"{name}T"
                )
            QT, KT, VT = proj_out["q"], proj_out["k"], proj_out["v"]

            # ---- input DMA, in consumption order ----
            # k weights+data first (gates everything), then the q slice for
            # q-block 0, then v (consumed by the interleaved V projection in
            # q-block 0's attention), then the rest of q, then wo.
            w_sb = {}
            b_sb = {}

            def issue_wb(name):
                w_t = wpool.tile([P, CC, DSL], f16, tag=f"w{name}")
                for wh in range(2):
                    nc.sync.dma_start(
                        w_t[:, :, wh * P : (wh + 1) * P],
                        w_r[name][:, :, wh * P : (wh + 1) * P],
                    )
                b_t = small.tile([P, DSL // P], f32, tag=f"b{name}")
                nc.sync.dma_start(b_t[:], b_r[name].rearrange("(o p) -> p o", p=P))
                w_sb[name], b_sb[name] = w_t, b_t

            es_k = ExitStack()
            xkpool = es_k.enter_context(tc.tile_pool(name="xk", bufs=1))
            xq_tiles = {}  # (quarter, qb) -> [P, 2, QB] tile
            xq_stacks = {}
            xk_tiles = []
            xv_tiles = []

            def issue_xq(qb_):
                st = ExitStack()
                pool = st.enter_context(tc.tile_pool(name=f"xqp{qb_}", bufs=1))
                for qt in range(4):
                    t = pool.tile([P, 2, QB], f16, tag=f"xq{qt}")
                    nc.sync.dma_start(
                        t[:],
                        x_r["q"][:, 2 * qt : 2 * qt + 2,
                                 qb_ * QB : (qb_ + 1) * QB],
                    )
                    xq_tiles[(qt, qb_)] = t
                xq_stacks[qb_] = st

            issue_wb("k")
            for qt in range(4):
                t = xkpool.tile([P, 2, S], f16, tag=f"xk{qt}")
                nc.sync.dma_start(t[:], x_r["k"][:, 2 * qt : 2 * qt + 2, :])
                xk_tiles.append(t)
            issue_wb("q")
            issue_xq(0)
            issue_wb("v")
            woT_sb = wpool.tile([P, DSL // P, HIDDEN], f16, tag="wo")
            nc.sync.dma_start(woT_sb[:], woT_d.rearrange("(c p) e -> p c e", p=P))

            # ---- K projection + q-block-0 Q projection (own PSUM scope) ----
            with tc.tile_pool(name="proj_ps", bufs=4, space="PSUM") as proj_ps:
                for mc in range(DSL // P):
                    pss = [
                        proj_ps.tile([P, 512], f32, tag="proj", name=f"kp{mc}{ns}")
                        for ns in range(4)
                    ]
                    for cc in range(CC):
                        for ns in range(4):
                            nc.tensor.matmul(
                                pss[ns][:],
                                w_sb["k"][:, cc, mc * P : (mc + 1) * P],
                                xk_tiles[cc // 2][:, cc % 2,
                                                  ns * 512 : (ns + 1) * 512],
                                start=(cc == 0),
                                stop=(cc == CC - 1),
                            )
                    for ns in range(4):
                        nc.vector.tensor_scalar_add(
                            KT[:, mc, ns * 512 : (ns + 1) * 512],
                            pss[ns][:],
                            b_sb["k"][:, mc : mc + 1],
                        )
                # Q projection for q-block 0
                for mc in range(DSL // P):
                    ps = proj_ps.tile([P, QB], f32, tag="proj", name=f"qp{mc}")
                    for cc in range(CC):
                        nc.tensor.matmul(
                            ps[:],
                            w_sb["q"][:, cc, mc * P : (mc + 1) * P],
                            xq_tiles[(cc // 2, 0)][:, cc % 2, :],
                            start=(cc == 0),
                            stop=(cc == CC - 1),
                        )
                    nc.vector.tensor_scalar_add(
                        QT[:, mc, 0:QB], ps[:], b_sb["q"][:, mc : mc + 1]
                    )
            xq_stacks[0].close()
            es_k.close()

            vprime = [None] * HPG
            XT = xT_pool.tile([P, DSL // P, S], f16, tag="XT")

            # ---- attention with interleaved projections ----
            with (
                tc.tile_pool(name="expT", bufs=2) as exp_pool,
                tc.tile_pool(name="norm", bufs=2) as norm_pool,
                tc.tile_pool(name="y_out", bufs=1) as ypool,
                tc.tile_pool(name="sc_ps", bufs=2, space="PSUM") as sc_ps,
                tc.tile_pool(name="acc_ps", bufs=2, space="PSUM") as acc_ps,
                tc.tile_pool(name="rby_ps", bufs=2, space="PSUM") as rby_ps,
            ):

                def emit_vproj(mc, ns):
                    # V-projection filler: one ns-chunk (8 matmuls) + bias
                    ps = rby_ps.tile([P, 512], f32, tag="rby",
                                     name=f"vp{mc}{ns}")
                    for cc in range(CC):
                        nc.tensor.matmul(
                            ps[:],
                            w_sb["v"][:, cc, mc * P : (mc + 1) * P],
                            xv_tiles[cc // 2][:, cc % 2,
                                              ns * 512 : (ns + 1) * 512],
                            start=(cc == 0),
                            stop=(cc == CC - 1),
                        )
                    nc.vector.tensor_scalar_add(
                        VT[:, mc, ns * 512 : (ns + 1) * 512],
                        ps[:],
                        b_sb["v"][:, mc : mc + 1],
                    )

                def emit_vprime(hs):
                    # V' build filler: PE-transpose VT 64x128 blocks into
                    # [s, d] tiles with a ones column (row-sum trick).
                    for h in hs:
                        vp = xT_pool.tile([P, KC, D_K + 1], f16, tag=f"vp{h}")
                        nc.vector.memset(vp[:], 1.0)
                        hc, hp = divmod(h, 2)
                        pb = hp * D_K
                        idn = ident[pb : pb + D_K, pb : pb + D_K]
                        for kc4 in range(KC // 4):
                            tp = rby_ps.tile([P, 4, D_K], f16, tag="rby",
                                             name=f"vt{h}{kc4}")
                            for j in range(4):
                                kc = kc4 * 4 + j
                                nc.tensor.transpose(
                                    tp[:, j, :],
                                    VT[pb : pb + D_K, hc,
                                       kc * P : (kc + 1) * P],
                                    idn,
                                )
                            nc.vector.tensor_copy(
                                vp[:, kc4 * 4 : kc4 * 4 + 4, 0:D_K], tp[:]
                            )
                        vprime[h] = vp

                def emit_qproj(qb_, mc):
                    ps = rby_ps.tile([P, QB], f32, tag="rby",
                                     name=f"qp{qb_}{mc}")
                    for cc in range(CC):
                        nc.tensor.matmul(
                            ps[:],
                            w_sb["q"][:, cc, mc * P : (mc + 1) * P],
                            xq_tiles[(cc // 2, qb_)][:, cc % 2, :],
                            start=(cc == 0),
                            stop=(cc == CC - 1),
                        )
                    nc.vector.tensor_scalar_add(
                        QT[:, mc, qb_ * QB : (qb_ + 1) * QB],
                        ps[:],
                        b_sb["q"][:, mc : mc + 1],
                    )

                def emit_norm_late(ctx):
                    # broadcast the approx-reciprocal row across partitions
                    # with a K=1 fp16 matmul, then scale the unnormalized
                    # head outputs.
                    for h, qb_, xun, rec16 in ctx:
                        hc, hp = divmod(h, 2)
                        qs_ = slice(qb_ * QB, (qb_ + 1) * QB)
                        rb_ps = rby_ps.tile([D_K, QB], f32, tag="rby",
                                            name=f"rb{h}")
                        nc.tensor.matmul(
                            rb_ps[:],
                            ones16[D_K : D_K + 1, :],
                            rec16[D_K : D_K + 1, :],
                            start=True,
                            stop=True,
                        )
                        if hp == 0:
                            nc.vector.tensor_tensor(
                                XT[0:D_K, hc, qs_], xun[:], rb_ps[:],
                                mybir.AluOpType.mult,
                            )
                        else:
                            tmp = norm_pool.tile([D_K, QB], f16, tag="xtmp")
                            nc.vector.tensor_tensor(
                                tmp[:], xun[:], rb_ps[:],
                                mybir.AluOpType.mult,
                            )
                            nc.sync.dma_start(XT[D_K:P, hc, qs_], tmp[:])

                def emit_outproj_half(qb_, half, y_sb):
                    for sc4 in (2 * half, 2 * half + 1):
                        sc = qb_ * 4 + sc4
                        ps2 = [
                            rby_ps.tile([P, 512], f32, tag="rby",
                                        name=f"yp{sc4}{ec}")
                            for ec in range(2)
                        ]
                        for dc in range(DSL // P):
                            for ec in range(2):
                                nc.tensor.matmul(
                                    ps2[ec][:],
                                    XT[:, dc, sc * P : (sc + 1) * P],
                                    woT_sb[:, dc, ec * 512 : (ec + 1) * 512],
                                    start=(dc == 0),
                                    stop=(dc == DSL // P - 1),
                                )
                        for ec in range(2):
                            nc.vector.tensor_copy(
                                y_sb[:, sc4, ec * 512 : (ec + 1) * 512],
                                ps2[ec][:],
                            )

                def emit_epilogue(heads, qb_, accs):
                    # drain acc PSUM right away: Pool copies the unnormalized
                    # output and the f32->f16 reciprocal row; DVE does the
                    # single-pass approx reciprocal.
                    ctx = []
                    for h in heads:
                        acc = accs[h]
                        xun = norm_pool.tile([D_K, QB], f32, tag="xun",
                                             name=f"xun{h}")
                        nc.vector.tensor_copy(xun[:], acc[0:D_K, :])
                        # 1/sum = exp(-ln(sum)) on the scalar engine; the
                        # explicit natural_log_exp_and_others table load above
                        # serves both ln and the attention exps (no reloads),
                        # and this takes the 3.35us multi-pass DVE reciprocal
                        # off the pair-boundary critical chain.
                        lnr = norm_pool.tile([D_K + 1, QB], f32, tag="lnr",
                                             name=f"ln{h}")
                        nc.scalar.activation(
                            lnr[D_K : D_K + 1, :],
                            acc[D_K : D_K + 1, :],
                            mybir.ActivationFunctionType.Ln,
                        )
                        rec16 = norm_pool.tile([D_K + 1, QB], f16, tag="rec16",
                                               name=f"rc{h}")
                        nc.scalar.activation(
                            rec16[D_K : D_K + 1, :],
                            lnr[D_K : D_K + 1, :],
                            mybir.ActivationFunctionType.Exp,
                            scale=-1.0,
                        )
                        ctx.append((h, qb_, xun, rec16))
                    return ctx

                def emit_scores(heads, qb_, g, expm):
                    qs = slice(qb_ * QB, (qb_ + 1) * QB)
                    for h in heads:
                        hc, hp = divmod(h, 2)
                        pb = hp * D_K
                        scs = sc_ps.tile([P, 2, QB], f32, tag="sc",
                                         name=f"sc{h}{g}")
                        for j in range(2):
                            kc = 2 * g + j
                            nc.tensor.matmul(
                                scs[:, j, :],
                                KT[pb : pb + D_K, hc, kc * P : (kc + 1) * P],
                                QT[pb : pb + D_K, hc, qs],
                                start=True,
                                stop=True,
                                tile_position=(pb, 0),
                            )
                        nc.scalar.activation(
                            expm[:, 2 * g : 2 * g + 2, hp, :],
                            scs[:],
                            mybir.ActivationFunctionType.Exp,
                            scale=float(SCALE),
                        )

                def emit_attnv(heads, accs, expm, kcs):
                    for kc in kcs:
                        for h in heads:
                            hp = h & 1
                            nc.tensor.matmul(
                                accs[h][:],
                                vprime[h][:, kc, :],
                                expm[:, kc, hp, :],
                                start=(kc == 0),
                                stop=(kc == KC - 1),
                            )

                pending_norm = None
                pending_outproj = None

                es_v = ExitStack()
                xvpool = es_v.enter_context(tc.tile_pool(name="xv", bufs=1))
                for qt in range(4):
                    t = xvpool.tile([P, 2, S], f16, tag=f"xv{qt}")
                    nc.sync.dma_start(t[:], x_r["v"][:, 2 * qt : 2 * qt + 2, :])
                    xv_tiles.append(t)
                issue_xq(1)

                # ---- q-block 0, pair 0: scores only, V filler ----
                heads0 = (0, 1)
                expm0 = exp_pool.tile([P, KC, 2, QB], f16, tag="exp",
                                      name="ex00")
                for g in range(NG):
                    if g >= 2 and g <= 5:
                        emit_vproj(0, g - 2)
                    elif g == 6:
                        emit_vprime((0, 1))
                    elif g == 7:
                        emit_vproj(1, 0)
                        emit_vproj(1, 1)
                    emit_scores(heads0, 0, g, expm0)

                # ---- q-block 0, pair 1: pair-0 attn@V as filler, D=4 ----
                heads1 = (2, 3)
                expm1 = exp_pool.tile([P, KC, 2, QB], f16, tag="exp",
                                      name="ex01")
                accs0 = {h: acc_ps.tile([D_K + 1, QB], f32, tag="acc",
                                        name=f"acc{h}") for h in heads0}
                accs1 = {}
                D1 = 4
                for g in range(NG + D1):
                    if g == 0:
                        emit_vproj(1, 2)
                        emit_vproj(1, 3)
                    elif g == 1:
                        emit_vprime((2, 3))
                    if g < 4:
                        emit_attnv(heads0, accs0, expm0, range(4 * g, 4 * g + 4))
                        if g == 3:
                            pending_norm = emit_epilogue(heads0, 0, accs0)
                    if g == D1:
                        for h in heads1:
                            accs1[h] = acc_ps.tile([D_K + 1, QB], f32,
                                                   tag="acc", name=f"acc{h}")
                    if g >= D1:
                        emit_attnv(heads1, accs1, expm1,
                                   (2 * (g - D1), 2 * (g - D1) + 1))
                    if g == 6 and pending_norm is not None:
                        emit_norm_late(pending_norm)
                        pending_norm = None
                    if g == 8:
                        emit_qproj(1, 0)
                    elif g == 9:
                        emit_qproj(1, 1)
                        xq_stacks[1].close()
                        es_v.close()
                        issue_xq(2)
                    if g < NG:
                        emit_scores(heads1, 0, g, expm1)
                pending_norm = emit_epilogue(heads1, 0, accs1)
                pending_outproj = 0

                # ---- steady state: q-blocks 1..3 ----
                # D=3: the last attn@V group's exp dependency completes while
                # the two preceding attnv-only groups run, so the in-order PE
                # never stalls at the pair tail (the stall was re-triggering
                # the HAM half-clock).
                D2 = 3
                for qb in range(1, N_QB):
                    for hpair in range(2):
                        heads = (2 * hpair, 2 * hpair + 1)
                        expm = exp_pool.tile([P, KC, 2, QB], f16, tag="exp",
                                             name=f"ex{qb}{hpair}")
                        accs = {}
                        for g in range(NG + D2):
                            if g == D2:
                                for h in heads:
                                    accs[h] = acc_ps.tile(
                                        [D_K + 1, QB], f32, tag="acc",
                                        name=f"acc{h}"
                                    )
                            if g >= D2:
                                emit_attnv(heads, accs, expm,
                                           (2 * (g - D2), 2 * (g - D2) + 1))
                            if g == 2 and pending_norm is not None:
                                emit_norm_late(pending_norm)
                                pending_norm = None
                            if g == 4 and hpair == 0 and \
                                    pending_outproj is not None:
                                y_sb = ypool.tile([P, 4, HIDDEN], f16,
                                                  tag="y", name=f"y{qb}")
                                emit_outproj_half(pending_outproj, 0, y_sb)
                            elif g == 5 and hpair == 0 and \
                                    pending_outproj is not None:
                                emit_outproj_half(pending_outproj, 1, y_sb)
                                nc.sync.dma_start(
                                    y_r[:, pending_outproj * 4 :
                                        pending_outproj * 4 + 4, :],
                                    y_sb[:],
                                )
                                pending_outproj = None
                            elif g == 6 and hpair == 1 and qb < N_QB - 1:
                                emit_qproj(qb + 1, 0)
                            elif g == 7 and hpair == 1 and qb < N_QB - 1:
                                emit_qproj(qb + 1, 1)
                                xq_stacks[qb + 1].close()
                                if qb + 2 < N_QB:
                                    issue_xq(qb + 2)
                            if g < NG:
                                emit_scores(heads, qb, g, expm)
                        pending_norm = emit_epilogue(heads, qb, accs)
                    pending_outproj = qb

                # tail
                emit_norm_late(pending_norm)
                y_sb = ypool.tile([P, 4, HIDDEN], f16, tag="y", name="ytail")
                emit_outproj_half(pending_outproj, 0, y_sb)
                emit_outproj_half(pending_outproj, 1, y_sb)
                nc.sync.dma_start(
                    y_r[:, pending_outproj * 4 : pending_outproj * 4 + 4, :],
                    y_sb[:],
                )

    nc.finalize()
    return nc


_NC_CACHE = None


def _get_nc():
    global _NC_CACHE
    if _NC_CACHE is None:
        _NC_CACHE = _build_nc()
    return _NC_CACHE


def make_in_maps(q, k, v, Wq, bq, Wk, bk, Wv, bv, Wo):
    """Host-side sharding: per-core input dicts (core = b * G + g)."""
    f16 = np.float16
    qT = [np.ascontiguousarray(q[b].T).astype(f16) for b in range(B)]
    kT = [np.ascontiguousarray(k[b].T).astype(f16) for b in range(B)]
    vT = [np.ascontiguousarray(v[b].T).astype(f16) for b in range(B)]
    in_maps = []
    for core in range(B * G):
        b, g = divmod(core, G)
        sl = slice(g * DSL, (g + 1) * DSL)
        in_maps.append(
            {
                "qT": qT[b],
                "kT": kT[b],
                "vT": vT[b],
                "wqT": np.ascontiguousarray(Wq[sl, :].T).astype(f16),
                "wkT": np.ascontiguousarray(Wk[sl, :].T).astype(f16),
                "wvT": np.ascontiguousarray(Wv[sl, :].T).astype(f16),
                "woT": np.ascontiguousarray(Wo[:, sl].T).astype(f16),
                "bq": np.ascontiguousarray(bq[sl], np.float32),
                "bk": np.ascontiguousarray(bk[sl], np.float32),
                "bv": np.ascontiguousarray(bv[sl], np.float32),
            }
        )
    return in_maps


def kernel(q, k, v, Wq, bq, Wk, bk, Wv, bv, Wo, bo):
    from concourse.bass_utils import run_bass_kernel_spmd

    q, k, v = (np.asarray(a, np.float32) for a in (q, k, v))
    Wq, Wk, Wv, Wo = (np.asarray(a, np.float32) for a in (Wq, Wk, Wv, Wo))
    bq, bk, bv, bo = (np.asarray(a, np.float32) for a in (bq, bk, bv, bo))

    nc = _get_nc()
    in_maps = make_in_maps(q, k, v, Wq, bq, Wk, bk, Wv, bv, Wo)
    res = run_bass_kernel_spmd(nc, in_maps, core_ids=list(range(B * G)))

    out = np.zeros((B, S, HIDDEN), np.float32)
    for b in range(B):
        acc = np.zeros((S, HIDDEN), np.float32)
        for g in range(G):
            acc += res.results[b * G + g]["y"].astype(np.float32)
        out[b] = acc + bo
    return out


# revision 24
# speedup vs baseline: 1.0445x; 1.0445x over previous
"""Trainium2 Bass kernel for nn_MultiHeadAttention (B=2, S=2048, H=1024, 16 heads).

Sharding: 8 cores = 2 (batch) x 4 (head-groups of 4 heads). Each core computes
QKV projections for its 256-dim head slice, attention for its 4 heads, and a
partial output projection. Host sums the 4 head-group partials per batch and
adds the output bias.

On-chip layout: activations live transposed as [d, s] with the hidden/head dim
on partitions, so every matmul contraction runs on the PE partition axis with
no activation transposes (inputs are pre-transposed on the host during
sharding). Attention uses unnormalized exp scores with a fused ones-column in
V to produce row sums, normalizing the small [64, S] per-head output instead
of the [S, S] attention matrix.

Schedule: the steady-state attention loop is gated by the scalar engine's exp
throughput, and the PE HAM governor halves the PE clock whenever the PE
micro-idles. So (1) within each group the trailing attn@V and any filler
matmuls are emitted BEFORE the scores matmuls, so the in-order PE queue always
has eligible work while the scores wait on the exp PSUM rotation, and (2) the
V projection, V' transposes and per-q-block Q projections are interleaved into
the attention stream as PE filler instead of running as separate phases (this
also hides the input DMA: scores only need K and the first Q block). Small
copies run on the otherwise-idle Pool engine; softmax normalization uses the
single-pass reciprocal_approx_fast and an fp16 K=1 broadcast matmul.
"""

import sys

if "/opt/trn_rl_repo" not in sys.path:
    sys.path.insert(0, "/opt/trn_rl_repo")

import numpy as np

HIDDEN, HEADS, D_K, B, S = 1024, 16, 64, 2, 2048
G = 4              # head groups (tensor-parallel dim)
HPG = HEADS // G   # heads per group
DSL = HPG * D_K    # 256: d-slice per core
P = 128
QB = 512           # q-block size for attention tiling
N_QB = S // QB     # 4
KC = S // P        # 16 k-chunks
NG = KC // 2       # 8 two-chunk groups
CC = HIDDEN // P   # 8 contraction chunks for projections
SCALE = 1.0 / np.sqrt(np.float32(D_K))


def _build_nc():
    from contextlib import ExitStack

    import concourse.mybir as mybir
    import concourse.tile as tile
    from concourse.bacc import Bacc
    from concourse.masks import make_identity

    dt = mybir.dt
    f32 = dt.float32
    f16 = dt.float16

    nc = Bacc(None)

    qT_d = nc.dram_tensor("qT", [HIDDEN, S], f16, kind="ExternalInput")
    kT_d = nc.dram_tensor("kT", [HIDDEN, S], f16, kind="ExternalInput")
    vT_d = nc.dram_tensor("vT", [HIDDEN, S], f16, kind="ExternalInput")
    wqT_d = nc.dram_tensor("wqT", [HIDDEN, DSL], f16, kind="ExternalInput")
    wkT_d = nc.dram_tensor("wkT", [HIDDEN, DSL], f16, kind="ExternalInput")
    wvT_d = nc.dram_tensor("wvT", [HIDDEN, DSL], f16, kind="ExternalInput")
    woT_d = nc.dram_tensor("woT", [DSL, HIDDEN], f16, kind="ExternalInput")
    bq_d = nc.dram_tensor("bq", [DSL], f32, kind="ExternalInput")
    bk_d = nc.dram_tensor("bk", [DSL], f32, kind="ExternalInput")
    bv_d = nc.dram_tensor("bv", [DSL], f32, kind="ExternalInput")
    y_d = nc.dram_tensor("y", [S, HIDDEN], f16, kind="ExternalOutput")
    y_r = y_d.rearrange("(sc p) e -> p sc e", p=P)

    w_r = {
        "q": wqT_d.rearrange("(c p) d -> p c d", p=P),
        "k": wkT_d.rearrange("(c p) d -> p c d", p=P),
        "v": wvT_d.rearrange("(c p) d -> p c d", p=P),
    }
    b_r = {"q": bq_d, "k": bk_d, "v": bv_d}
    x_r = {
        "q": qT_d.rearrange("(c p) s -> p c s", p=P),
        "k": kT_d.rearrange("(c p) s -> p c s", p=P),
        "v": vT_d.rearrange("(c p) s -> p c s", p=P),
    }

    with tile.TileContext(nc) as tc:
        with (
            tc.tile_pool(name="weights", bufs=1) as wpool,
            tc.tile_pool(name="qkvT", bufs=1) as qkvT_pool,
            tc.tile_pool(name="xT_out", bufs=1) as xT_pool,
            tc.tile_pool(name="small", bufs=1) as small,
        ):
            nc.scalar.add_instruction(
                mybir.InstLoadActFuncSet(
                    name=nc.get_next_instruction_name(),
                    ins=[],
                    outs=[],
                    act_func_set_id=6,  # natural_log_exp_and_others
                )
            )
            ident = small.tile([P, P], f16)
            make_identity(nc, ident)
            ones16 = small.tile([P, D_K], f16, tag="ones")
            nc.vector.memset(ones16[:], 1.0)
            ones32 = small.tile([P, D_K], f32, tag="ones32")
            nc.vector.memset(ones32[:], 1.0)

            proj_out = {}
            for name in ("k", "v", "q"):
                proj_out[name] = qkvT_pool.tile(
                    [P, DSL // P, S], f16, tag=f"{name}T", name=f"{name}T"
                )
            QT, KT, VT = proj_out["q"], proj_out["k"], proj_out["v"]

            # ---- input DMA, in consumption order ----
            # k weights+data first (gates everything), then the q slice for
            # q-block 0, then v (consumed by the interleaved V projection in
            # q-block 0's attention), then the rest of q, then wo.
            w_sb = {}
            b_sb = {}

            def issue_wb(name):
                w_t = wpool.tile([P, CC, DSL], f16, tag=f"w{name}")
                for wh in range(2):
                    nc.sync.dma_start(
                        w_t[:, :, wh * P : (wh + 1) * P],
                        w_r[name][:, :, wh * P : (wh + 1) * P],
                    )
                b_t = small.tile([P, DSL // P], f32, tag=f"b{name}")
                nc.sync.dma_start(b_t[:], b_r[name].rearrange("(o p) -> p o", p=P))
                w_sb[name], b_sb[name] = w_t, b_t

            es_k = ExitStack()
            xkpool = es_k.enter_context(tc.tile_pool(name="xk", bufs=1))
            xq_tiles = {}  # (quarter, qb) -> [P, 2, QB] tile
            xq_stacks = {}
            xk_tiles = []
            xv_tiles = []

            def issue_xq(qb_):
                st = ExitStack()
                pool = st.enter_context(tc.tile_pool(name=f"xqp{qb_}", bufs=1))
                for qt in range(4):
                    t = pool.tile([P, 2, QB], f16, tag=f"xq{qt}")
                    nc.sync.dma_start(
                        t[:],
                        x_r["q"][:, 2 * qt : 2 * qt + 2,
                                 qb_ * QB : (qb_ + 1) * QB],
                    )
                    xq_tiles[(qt, qb_)] = t
                xq_stacks[qb_] = st

            issue_wb("k")
            for qt in range(4):
                t = xkpool.tile([P, 2, S], f16, tag=f"xk{qt}")
                nc.sync.dma_start(t[:], x_r["k"][:, 2 * qt : 2 * qt + 2, :])
                xk_tiles.append(t)
            issue_wb("q")
            issue_xq(0)
            issue_wb("v")
            woT_sb = wpool.tile([P, DSL // P, HIDDEN], f16, tag="wo")
            nc.sync.dma_start(woT_sb[:], woT_d.rearrange("(c p) e -> p c e", p=P))

            # ---- K projection + q-block-0 Q projection (own PSUM scope) ----
            with tc.tile_pool(name="proj_ps", bufs=4, space="PSUM") as proj_ps:
                for mc in range(DSL // P):
                    pss = [
                        proj_ps.tile([P, 512], f32, tag="proj", name=f"kp{mc}{ns}")
                        for ns in range(4)
                    ]
                    for cc in range(CC):
                        for ns in range(4):
                            nc.tensor.matmul(
                                pss[ns][:],
                                w_sb["k"][:, cc, mc * P : (mc + 1) * P],
                                xk_tiles[cc // 2][:, cc % 2,
                                                  ns * 512 : (ns + 1) * 512],
                                start=(cc == 0),
                                stop=(cc == CC - 1),
                            )
                    for ns in range(4):
                        nc.vector.tensor_scalar_add(
                            KT[:, mc, ns * 512 : (ns + 1) * 512],
                            pss[ns][:],
                            b_sb["k"][:, mc : mc + 1],
                        )
                # Q projection for q-block 0
                for mc in range(DSL // P):
                    ps = proj_ps.tile([P, QB], f32, tag="proj", name=f"qp{mc}")
                    for cc in range(CC):
                        nc.tensor.matmul(
                            ps[:],
                            w_sb["q"][:, cc, mc * P : (mc + 1) * P],
                            xq_tiles[(cc // 2, 0)][:, cc % 2, :],
                            start=(cc == 0),
                            stop=(cc == CC - 1),
                        )
                    nc.vector.tensor_scalar_add(
                        QT[:, mc, 0:QB], ps[:], b_sb["q"][:, mc : mc + 1]
                    )
            xq_stacks[0].close()
            es_k.close()

            vprime = [None] * HPG
            XT = xT_pool.tile([P, DSL // P, S], f16, tag="XT")

            # ---- attention with interleaved projections ----
            with (
                tc.tile_pool(name="expT", bufs=2) as exp_pool,
                tc.tile_pool(name="norm", bufs=2) as norm_pool,
                tc.tile_pool(name="y_out", bufs=1) as ypool,
                tc.tile_pool(name="sc_ps", bufs=2, space="PSUM") as sc_ps,
                tc.tile_pool(name="acc_ps", bufs=2, space="PSUM") as acc_ps,
                tc.tile_pool(name="rby_ps", bufs=2, space="PSUM") as rby_ps,
            ):

                def emit_vproj(mc, ns):
                    # V-projection filler: one ns-chunk (8 matmuls) + bias
                    ps = rby_ps.tile([P, 512], f32, tag="rby",
                                     name=f"vp{mc}{ns}")
                    for cc in range(CC):
                        nc.tensor.matmul(
                            ps[:],
                            w_sb["v"][:, cc, mc * P : (mc + 1) * P],
                            xv_tiles[cc // 2][:, cc % 2,
                                              ns * 512 : (ns + 1) * 512],
                            start=(cc == 0),
                            stop=(cc == CC - 1),
                        )
                    nc.vector.tensor_scalar_add(
                        VT[:, mc, ns * 512 : (ns + 1) * 512],
                        ps[:],
                        b_sb["v"][:, mc : mc + 1],
                    )

                def emit_vprime(hs):
                    # V' build filler: PE-transpose VT 64x128 blocks into
                    # [s, d] tiles with a ones column (row-sum trick).
                    for h in hs:
                        vp = xT_pool.tile([P, KC, D_K + 1], f16, tag=f"vp{h}")
                        nc.vector.memset(vp[:], 1.0)
                        hc, hp = divmod(h, 2)
                        pb = hp * D_K
                        idn = ident[pb : pb + D_K, pb : pb + D_K]
                        for kc4 in range(KC // 4):
                            tp = rby_ps.tile([P, 4, D_K], f16, tag="rby",
                                             name=f"vt{h}{kc4}")
                            for j in range(4):
                                kc = kc4 * 4 + j
                                nc.tensor.transpose(
                                    tp[:, j, :],
                                    VT[pb : pb + D_K, hc,
                                       kc * P : (kc + 1) * P],
                                    idn,
                                )
                            nc.vector.tensor_copy(
                                vp[:, kc4 * 4 : kc4 * 4 + 4, 0:D_K], tp[:]
                            )
                        vprime[h] = vp

                def emit_qproj(qb_, mc):
                    ps = rby_ps.tile([P, QB], f32, tag="rby",
                                     name=f"qp{qb_}{mc}")
                    for cc in range(CC):
                        nc.tensor.matmul(
                            ps[:],
                            w_sb["q"][:, cc, mc * P : (mc + 1) * P],
                            xq_tiles[(cc // 2, qb_)][:, cc % 2, :],
                            start=(cc == 0),
                            stop=(cc == CC - 1),
                        )
                    nc.vector.tensor_scalar_add(
                        QT[:, mc, qb_ * QB : (qb_ + 1) * QB],
                        ps[:],
                        b_sb["q"][:, mc : mc + 1],
                    )

                def emit_norm_late(ctx):
                    # broadcast the approx-reciprocal row across partitions
                    # with a K=1 fp16 matmul, then scale the unnormalized
                    # head outputs.
                    for h, qb_, xun, rec16 in ctx:
                        hc, hp = divmod(h, 2)
                        qs_ = slice(qb_ * QB, (qb_ + 1) * QB)
                        rb_ps = rby_ps.tile([D_K, QB], f32, tag="rby",
                                            name=f"rb{h}")
                        nc.tensor.matmul(
                            rb_ps[:],
                            ones16[D_K : D_K + 1, :],
                            rec16[D_K : D_K + 1, :],
                            start=True,
                            stop=True,
                        )
                        if hp == 0:
                            nc.vector.tensor_tensor(
                                XT[0:D_K, hc, qs_], xun[:], rb_ps[:],
                                mybir.AluOpType.mult,
                            )
                        else:
                            tmp = norm_pool.tile([D_K, QB], f16, tag="xtmp")
                            nc.vector.tensor_tensor(
                                tmp[:], xun[:], rb_ps[:],
                                mybir.AluOpType.mult,
                            )
                            nc.sync.dma_start(XT[D_K:P, hc, qs_], tmp[:])

                def emit_outproj_half(qb_, half, y_sb):
                    for sc4 in (2 * half, 2 * half + 1):
                        sc = qb_ * 4 + sc4
                        ps2 = [
                            rby_ps.tile([P, 512], f32, tag="rby",
                                        name=f"yp{sc4}{ec}")
                            for ec in range(2)
                        ]
                        for dc in range(DSL // P):
                            for ec in range(2):
                                nc.tensor.matmul(
                                    ps2[ec][:],
                                    XT[:, dc, sc * P : (sc + 1) * P],
                                    woT_sb[:, dc, ec * 512 : (ec + 1) * 512],
                                    start=(dc == 0),
                                    stop=(dc == DSL // P - 1),
                                )
                        for ec in range(2):
                            nc.vector.tensor_copy(
                                y_sb[:, sc4, ec * 512 : (ec + 1) * 512],
                                ps2[ec][:],
                            )

                def emit_epilogue(heads, qb_, accs):
                    # drain acc PSUM right away: Pool copies the unnormalized
                    # output and the f32->f16 reciprocal row; DVE does the
                    # single-pass approx reciprocal.
                    ctx = []
                    for h in heads:
                        acc = accs[h]
                        xun = norm_pool.tile([D_K, QB], f32, tag="xun",
                                             name=f"xun{h}")
                        nc.vector.tensor_copy(xun[:], acc[0:D_K, :])
                        # 1/sum = exp(-ln(sum)) on the scalar engine; the
                        # explicit natural_log_exp_and_others table load above
                        # serves both ln and the attention exps (no reloads),
                        # and this takes the 3.35us multi-pass DVE reciprocal
                        # off the pair-boundary critical chain.
                        lnr = norm_pool.tile([D_K + 1, QB], f32, tag="lnr",
                                             name=f"ln{h}")
                        nc.scalar.activation(
                            lnr[D_K : D_K + 1, :],
                            acc[D_K : D_K + 1, :],
                            mybir.ActivationFunctionType.Ln,
                        )
                        rec16 = norm_pool.tile([D_K + 1, QB], f16, tag="rec16",
                                               name=f"rc{h}")
                        nc.scalar.activation(
                            rec16[D_K : D_K + 1, :],
                            lnr[D_K : D_K + 1, :],
                            mybir.ActivationFunctionType.Exp,
                            scale=-1.0,
                        )
                        ctx.append((h, qb_, xun, rec16))
                    return ctx

                def emit_scores(heads, qb_, g, expm):
                    # j-outer: the two heads' same-kc matmuls are adjacent
                    # and live in disjoint PE row groups (bases 0/64), giving
                    # the 64-deep reorder window a chance to co-issue them.
                    qs = slice(qb_ * QB, (qb_ + 1) * QB)
                    tiles = {}
                    for h in heads:
                        tiles[h] = sc_ps.tile([P, 2, QB], f32, tag="sc",
                                              name=f"sc{h}{g}")
                    for j in range(2):
                        kc = 2 * g + j
                        for h in heads:
                            hc, hp = divmod(h, 2)
                            pb = hp * D_K
                            nc.tensor.matmul(
                                tiles[h][:, j, :],
                                KT[pb : pb + D_K, hc, kc * P : (kc + 1) * P],
                                QT[pb : pb + D_K, hc, qs],
                                start=True,
                                stop=True,
                                tile_position=(pb, 0),
                            )
                    for h in heads:
                        hp = h & 1
                        nc.scalar.activation(
                            expm[:, 2 * g : 2 * g + 2, hp, :],
                            tiles[h][:],
                            mybir.ActivationFunctionType.Exp,
                            scale=float(SCALE),
                        )

                def emit_attnv(heads, accs, expm, kcs):
                    for kc in kcs:
                        for h in heads:
                            hp = h & 1
                            nc.tensor.matmul(
                                accs[h][:],
                                vprime[h][:, kc, :],
                                expm[:, kc, hp, :],
                                start=(kc == 0),
                                stop=(kc == KC - 1),
                            )

                pending_norm = None
                pending_outproj = None

                es_v = ExitStack()
                xvpool = es_v.enter_context(tc.tile_pool(name="xv", bufs=1))
                for qt in range(4):
                    t = xvpool.tile([P, 2, S], f16, tag=f"xv{qt}")
                    nc.sync.dma_start(t[:], x_r["v"][:, 2 * qt : 2 * qt + 2, :])
                    xv_tiles.append(t)
                issue_xq(1)

                # ---- q-block 0, pair 0: scores only, V filler ----
                heads0 = (0, 1)
                expm0 = exp_pool.tile([P, KC, 2, QB], f16, tag="exp",
                                      name="ex00")
                for g in range(NG):
                    if g >= 2 and g <= 5:
                        emit_vproj(0, g - 2)
                    elif g == 6:
                        emit_vprime((0, 1))
                    elif g == 7:
                        emit_vproj(1, 0)
                        emit_vproj(1, 1)
                    emit_scores(heads0, 0, g, expm0)

                # ---- q-block 0, pair 1: pair-0 attn@V as filler, D=4 ----
                heads1 = (2, 3)
                expm1 = exp_pool.tile([P, KC, 2, QB], f16, tag="exp",
                                      name="ex01")
                accs0 = {h: acc_ps.tile([D_K + 1, QB], f32, tag="acc",
                                        name=f"acc{h}") for h in heads0}
                accs1 = {}
                D1 = 4
                for g in range(NG + D1):
                    if g == 0:
                        emit_vproj(1, 2)
                        emit_vproj(1, 3)
                    elif g == 1:
                        emit_vprime((2, 3))
                    if g < 4:
                        emit_attnv(heads0, accs0, expm0, range(4 * g, 4 * g + 4))
                        if g == 3:
                            pending_norm = emit_epilogue(heads0, 0, accs0)
                    if g == D1:
                        for h in heads1:
                            accs1[h] = acc_ps.tile([D_K + 1, QB], f32,
                                                   tag="acc", name=f"acc{h}")
                    if g >= D1:
                        emit_attnv(heads1, accs1, expm1,
                                   (2 * (g - D1), 2 * (g - D1) + 1))
                    if g == 6 and pending_norm is not None:
                        emit_norm_late(pending_norm)
                        pending_norm = None
                    if g == 8:
                        emit_qproj(1, 0)
                    elif g == 9:
                        emit_qproj(1, 1)
                        xq_stacks[1].close()
                        es_v.close()
                        issue_xq(2)
                    if g < NG:
                        emit_scores(heads1, 0, g, expm1)
                pending_norm = emit_epilogue(heads1, 0, accs1)
                pending_outproj = 0

                # ---- steady state: q-blocks 1..3, D=2 ----
                D2 = 2
                for qb in range(1, N_QB):
                    for hpair in range(2):
                        heads = (2 * hpair, 2 * hpair + 1)
                        expm = exp_pool.tile([P, KC, 2, QB], f16, tag="exp",
                                             name=f"ex{qb}{hpair}")
                        accs = {}
                        for g in range(NG + D2):
                            if g == D2:
                                for h in heads:
                                    accs[h] = acc_ps.tile(
                                        [D_K + 1, QB], f32, tag="acc",
                                        name=f"acc{h}"
                                    )
                            if g >= D2:
                                emit_attnv(heads, accs, expm,
                                           (2 * (g - D2), 2 * (g - D2) + 1))
                            if g == 2 and pending_norm is not None:
                                emit_norm_late(pending_norm)
                                pending_norm = None
                            if g == 4 and hpair == 0 and \
                                    pending_outproj is not None:
                                y_sb = ypool.tile([P, 4, HIDDEN], f16,
                                                  tag="y", name=f"y{qb}")
                                emit_outproj_half(pending_outproj, 0, y_sb)
                            elif g == 5 and hpair == 0 and \
                                    pending_outproj is not None:
                                emit_outproj_half(pending_outproj, 1, y_sb)
                                nc.sync.dma_start(
                                    y_r[:, pending_outproj * 4 :
                                        pending_outproj * 4 + 4, :],
                                    y_sb[:],
                                )
                                pending_outproj = None
                            elif g == 6 and hpair == 1 and qb < N_QB - 1:
                                emit_qproj(qb + 1, 0)
                            elif g == 7 and hpair == 1 and qb < N_QB - 1:
                                emit_qproj(qb + 1, 1)
                                xq_stacks[qb + 1].close()
                                if qb + 2 < N_QB:
                                    issue_xq(qb + 2)
                            if g < NG:
                                emit_scores(heads, qb, g, expm)
                        pending_norm = emit_epilogue(heads, qb, accs)
                    pending_outproj = qb

                # tail
                emit_norm_late(pending_norm)
                y_sb = ypool.tile([P, 4, HIDDEN], f16, tag="y", name="ytail")
                emit_outproj_half(pending_outproj, 0, y_sb)
                emit_outproj_half(pending_outproj, 1, y_sb)
                nc.sync.dma_start(
                    y_r[:, pending_outproj * 4 : pending_outproj * 4 + 4, :],
                    y_sb[:],
                )

    nc.finalize()
    return nc


_NC_CACHE = None


def _get_nc():
    global _NC_CACHE
    if _NC_CACHE is None:
        _NC_CACHE = _build_nc()
    return _NC_CACHE


def make_in_maps(q, k, v, Wq, bq, Wk, bk, Wv, bv, Wo):
    """Host-side sharding: per-core input dicts (core = b * G + g)."""
    f16 = np.float16
    qT = [np.ascontiguousarray(q[b].T).astype(f16) for b in range(B)]
    kT = [np.ascontiguousarray(k[b].T).astype(f16) for b in range(B)]
    vT = [np.ascontiguousarray(v[b].T).astype(f16) for b in range(B)]
    in_maps = []
    for core in range(B * G):
        b, g = divmod(core, G)
        sl = slice(g * DSL, (g + 1) * DSL)
        in_maps.append(
            {
                "qT": qT[b],
                "kT": kT[b],
                "vT": vT[b],
                "wqT": np.ascontiguousarray(Wq[sl, :].T).astype(f16),
                "wkT": np.ascontiguousarray(Wk[sl, :].T).astype(f16),
                "wvT": np.ascontiguousarray(Wv[sl, :].T).astype(f16),
                "woT": np.ascontiguousarray(Wo[:, sl].T).astype(f16),
                "bq": np.ascontiguousarray(bq[sl], np.float32),
                "bk": np.ascontiguousarray(bk[sl], np.float32),
                "bv": np.ascontiguousarray(bv[sl], np.float32),
            }
        )
    return in_maps


def kernel(q, k, v, Wq, bq, Wk, bk, Wv, bv, Wo, bo):
    from concourse.bass_utils import run_bass_kernel_spmd

    q, k, v = (np.asarray(a, np.float32) for a in (q, k, v))
    Wq, Wk, Wv, Wo = (np.asarray(a, np.float32) for a in (Wq, Wk, Wv, Wo))
    bq, bk, bv, bo = (np.asarray(a, np.float32) for a in (bq, bk, bv, bo))

    nc = _get_nc()
    in_maps = make_in_maps(q, k, v, Wq, bq, Wk, bk, Wv, bv, Wo)
    res = run_bass_kernel_spmd(nc, in_maps, core_ids=list(range(B * G)))

    out = np.zeros((B, S, HIDDEN), np.float32)
    for b in range(B):
        acc = np.zeros((S, HIDDEN), np.float32)
        for g in range(G):
            acc += res.results[b * G + g]["y"].astype(np.float32)
        out[b] = acc + bo
    return out
